# revision 10
# baseline (speedup 1.0000x reference)
"""Trainium2 Bass kernel for the GNN NodeModel problem.

Strategy:
  - Host: sort edges by destination (col), shard contiguously at node
    boundaries across 8 cores (no cross-core combine needed: each core
    owns a disjoint node range and all edges pointing into it).
  - Device phase 1 (edge MLP): one interleaved indirect-DMA gather of
    x[row],x[col] per 128 edges, PE-transpose to feature-major, 2-chunk
    matmul MLP1 (K=192), MLP2, relu+pad-mask, messages to DRAM scratch.
  - Device phase 2 (aggregate + node MLP): per 128-node block, padded
    indirect-DMA regather of messages (per-block max degree), free-dim
    reduce for sum, log-tree max, mean via 1/cnt, concat with x and
    u[batch], PE-transpose, MLP3/MLP4, output feature-major.
  - Three TileContexts (phase1 / phase2a / phase2b) so DMA semaphore
    counters reset — one context overflows the 16-bit wait immediates.
  - Host: concat per-core node ranges, transpose to [N, F_OUT].
"""

import sys

sys.path.insert(0, "/opt/trn_rl_repo")

import numpy as np

import concourse.bass as bass
import concourse.mybir as mybir
import concourse.tile as tile
from concourse.alu_op_type import AluOpType
from concourse.bass import IndirectOffsetOnAxis
from concourse.bass_utils import run_bass_kernel_spmd

N_CORES = 8
F = 64  # node/edge/msg/glob feature dim
HID = 32
F_OUT = 128
ET = 512  # edge tile (moving free dim)
P = 128

f32 = mybir.dt.float32
i32 = mybir.dt.int32
RELU = mybir.ActivationFunctionType.Relu


def split_multi_waits(nc):
    """This walrus build allows max 1 sync-wait per instruction; hoist
    extras onto same-engine NoOps inserted just before."""
    for f in nc.m.functions:
        for bb in f.blocks:
            new_insts = []
            for inst in bb.instructions:
                si = inst.sync_info
                waits = list(si.on_wait) if si and si.on_wait else []
                if len(waits) > 1:
                    for k, w in enumerate(waits[:-1]):
                        nop = mybir.InstNoOp(
                            name=f"{inst.name}-wsplit{k}",
                            engine=inst.engine,
                            ins=[],
                            outs=[],
                        )
                        nop.sync_info = mybir.SyncInfo(on_wait=[w], on_update=[])
                        new_insts.append(nop)
                    inst.sync_info = mybir.SyncInfo(
                        on_wait=[waits[-1]], on_update=list(si.on_update)
                    )
                new_insts.append(inst)
            bb.instructions[:] = new_insts


def build_program(n_nodes_total, e_pad, nodes_pad, d_max, db_list, b2_nonzero):
    """Build the SPMD Bass program. e_pad: padded edge count per core
    (multiple of ET, >= e_c+1). nodes_pad: padded node count per core
    (multiple of P). d_max: max regather depth; db_list: per-node-block
    regather depth (max over cores)."""
    nc = bass.Bass()
    n_etiles = e_pad // ET
    n_nblocks = nodes_pad // P
    t4 = n_etiles * 4  # 128-edge groups
    assert len(db_list) == n_nblocks

    x = nc.dram_tensor("x", [n_nodes_total, F], f32, kind="ExternalInput")
    rcT = nc.dram_tensor("rcT", [P, 2 * t4], i32, kind="ExternalInput")
    maskT = nc.dram_tensor("maskT", [P, t4], f32, kind="ExternalInput")
    ea = nc.dram_tensor("ea", [e_pad, F], f32, kind="ExternalInput")
    padidx = nc.dram_tensor("padidx", [nodes_pad, d_max], i32, kind="ExternalInput")
    cntinvT = nc.dram_tensor("cntinvT", [P, n_nblocks], f32, kind="ExternalInput")
    xw = nc.dram_tensor("xw", [nodes_pad, F], f32, kind="ExternalInput")
    ub = nc.dram_tensor("ub", [nodes_pad, F], f32, kind="ExternalInput")
    ident = nc.dram_tensor("ident", [P, P], f32, kind="ExternalInput")
    w1 = nc.dram_tensor("w1", [3 * F, HID], f32, kind="ExternalInput")
    b1 = nc.dram_tensor("b1", [HID, 1], f32, kind="ExternalInput")
    w2 = nc.dram_tensor("w2", [HID, F], f32, kind="ExternalInput")
    b2 = nc.dram_tensor("b2", [1, F], f32, kind="ExternalInput")
    w3 = nc.dram_tensor("w3", [4 * F, HID], f32, kind="ExternalInput")
    b3 = nc.dram_tensor("b3", [HID, 1], f32, kind="ExternalInput")
    w4 = nc.dram_tensor("w4", [HID, F_OUT], f32, kind="ExternalInput")
    b4 = nc.dram_tensor("b4", [F_OUT, 1], f32, kind="ExternalInput")

    outT = nc.dram_tensor("outT", [F_OUT, nodes_pad], f32, kind="ExternalOutput")
    m_dram = nc.dram_tensor("m_dram", [e_pad, F], f32, kind="Internal")

    # ---------------- phase 1: edge MLP ----------------
    with tile.TileContext(nc) as tc:
        with (
            tc.tile_pool(name="c1", bufs=1) as cpool,
            tc.tile_pool(name="sb1", bufs=3) as sb,
            tc.tile_pool(name="ps1", bufs=2, space="PSUM") as ps,
            tc.tile_pool(name="mps", bufs=2, space="PSUM") as mps,
        ):
            idn = cpool.tile([P, P], f32)
            nc.sync.dma_start(idn[:], ident[:])
            w1a = cpool.tile([2 * F, HID], f32)
            nc.sync.dma_start(w1a[:], w1[0 : 2 * F, :])
            w1b = cpool.tile([F, HID], f32)
            nc.sync.dma_start(w1b[:], w1[2 * F : 3 * F, :])
            b1t = cpool.tile([HID, 1], f32)
            nc.sync.dma_start(b1t[:], b1[:])
            w2t = cpool.tile([HID, F], f32)
            nc.sync.dma_start(w2t[:], w2[:])
            b2t = cpool.tile([1, F], f32)
            nc.sync.dma_start(b2t[:], b2[:])
            rcTt = cpool.tile([P, 2 * t4], i32)
            nc.sync.dma_start(rcTt[:], rcT[:])
            maskTt = cpool.tile([P, t4], f32)
            nc.sync.dma_start(maskTt[:], maskT[:])

            for it in range(n_etiles):
                e0 = it * ET
                xallT = sb.tile([P, ET], f32, tag="xallT")
                eaT = sb.tile([F, ET], f32, tag="eaT")
                for j in range(4):
                    gidx = 4 * it + j
                    g = sb.tile([P, 2 * F], f32, tag="g")
                    nc.gpsimd.indirect_dma_start(
                        out=g[:, 0:F],
                        out_offset=None,
                        in_=x[:, :],
                        in_offset=IndirectOffsetOnAxis(
                            ap=rcTt[:, 2 * gidx : 2 * gidx + 1], axis=0
                        ),
                    )
                    nc.gpsimd.indirect_dma_start(
                        out=g[:, F : 2 * F],
                        out_offset=None,
                        in_=x[:, :],
                        in_offset=IndirectOffsetOnAxis(
                            ap=rcTt[:, 2 * gidx + 1 : 2 * gidx + 2], axis=0
                        ),
                    )
                    gT = ps.tile([P, P], f32, tag="tps", space="PSUM")
                    nc.tensor.transpose(out=gT[:], in_=g[:], identity=idn[:])
                    if j % 2 == 0:
                        nc.vector.tensor_copy(
                            out=xallT[:, j * P : (j + 1) * P], in_=gT[:]
                        )
                    else:
                        nc.scalar.copy(out=xallT[:, j * P : (j + 1) * P], in_=gT[:])

                    gea = sb.tile([P, F], f32, tag="gea")
                    nc.sync.dma_start(gea[:], ea[e0 + j * P : e0 + (j + 1) * P, :])
                    geaT = ps.tile([P, P], f32, tag="tps", space="PSUM")
                    nc.tensor.transpose(out=geaT[:F, :], in_=gea[:], identity=idn[:])
                    if j % 2 == 0:
                        nc.scalar.copy(out=eaT[:, j * P : (j + 1) * P], in_=geaT[:F, :])
                    else:
                        nc.vector.tensor_copy(
                            out=eaT[:, j * P : (j + 1) * P], in_=geaT[:F, :]
                        )

                hps = ps.tile([HID, ET], f32, tag="hps", space="PSUM")
                nc.tensor.matmul(
                    out=hps[:], lhsT=w1a[:], rhs=xallT[:], start=True, stop=False
                )
                nc.tensor.matmul(
                    out=hps[:], lhsT=w1b[:], rhs=eaT[:], start=False, stop=True
                )
                hsb = sb.tile([HID, ET], f32, tag="hsb")
                nc.scalar.activation(
                    out=hsb[:], in_=hps[:], func=RELU, bias=b1t[:, :1]
                )
                for j in range(4):
                    mp = mps.tile([P, F], f32, tag="mp", space="PSUM")
                    nc.tensor.matmul(
                        out=mp[:],
                        lhsT=hsb[:, j * P : (j + 1) * P],
                        rhs=w2t[:],
                        start=True,
                        stop=True,
                    )
                    msb = sb.tile([P, F], f32, tag="msb")
                    if b2_nonzero:
                        madd = sb.tile([P, F], f32, tag="madd")
                        nc.vector.tensor_add(
                            madd[:], mp[:], b2t[:].to_broadcast([P, F])
                        )
                        nc.scalar.activation(out=msb[:], in_=madd[:], func=RELU)
                    else:
                        nc.scalar.activation(out=msb[:], in_=mp[:], func=RELU)
                    mz = sb.tile([P, F], f32, tag="mz")
                    nc.vector.tensor_scalar_mul(
                        mz[:], msb[:], maskTt[:, 4 * it + j : 4 * it + j + 1]
                    )
                    nc.sync.dma_start(m_dram[e0 + j * P : e0 + (j + 1) * P, :], mz[:])

    # ---------------- phase 2: aggregate + node MLP (two contexts) ----
    halves = [(0, n_nblocks // 2), (n_nblocks // 2, n_nblocks)]
    for lo, hi in halves:
        if lo == hi:
            continue
        with tile.TileContext(nc) as tc:
            with (
                tc.tile_pool(name=f"c2_{lo}", bufs=1) as cpool,
                tc.tile_pool(name=f"sb2_{lo}", bufs=3) as sb,
                tc.tile_pool(name=f"ps2_{lo}", bufs=2, space="PSUM") as ps,
            ):
                idn = cpool.tile([P, P], f32)
                nc.sync.dma_start(idn[:], ident[:])
                w3a = cpool.tile([2 * F, HID], f32)
                nc.sync.dma_start(w3a[:], w3[0 : 2 * F, :])
                w3b = cpool.tile([2 * F, HID], f32)
                nc.sync.dma_start(w3b[:], w3[2 * F : 4 * F, :])
                b3t = cpool.tile([HID, 1], f32)
                nc.sync.dma_start(b3t[:], b3[:])
                w4t = cpool.tile([HID, F_OUT], f32)
                nc.sync.dma_start(w4t[:], w4[:])
                b4t = cpool.tile([F_OUT, 1], f32)
                nc.sync.dma_start(b4t[:], b4[:])
                cntTt = cpool.tile([P, n_nblocks], f32)
                nc.sync.dma_start(cntTt[:], cntinvT[:])

                for nb in range(lo, hi):
                    n0 = nb * P
                    db = db_list[nb]
                    pidx = sb.tile([P, d_max], i32, tag="pidx")
                    nc.sync.dma_start(pidx[:, :db], padidx[n0 : n0 + P, 0:db])

                    macc = sb.tile([P, d_max * F], f32, tag="macc")
                    for d in range(db):
                        nc.gpsimd.indirect_dma_start(
                            out=macc[:, d * F : (d + 1) * F],
                            out_offset=None,
                            in_=m_dram[:, :],
                            in_offset=IndirectOffsetOnAxis(
                                ap=pidx[:, d : d + 1], axis=0
                            ),
                        )
                    mview = macc[:, 0 : db * F].rearrange("p (d f) -> p f d", f=F)
                    hcat = sb.tile([P, 4 * F], f32, tag="hcat")
                    nc.sync.dma_start(hcat[:, 0:F], xw[n0 : n0 + P, :])
                    asum = sb.tile([P, F], f32, tag="asum")
                    nc.vector.tensor_reduce(
                        out=asum[:], in_=mview, axis=mybir.AxisListType.X,
                        op=AluOpType.add,
                    )
                    nc.vector.tensor_scalar_mul(
                        hcat[:, F : 2 * F], asum[:], cntTt[:, nb : nb + 1]
                    )
                    # segment max: log-tree of elementwise max (DVE),
                    # runs after the sum reduce via WAR dependency
                    cur = db
                    while cur > 1:
                        h = cur // 2
                        nc.vector.tensor_max(
                            macc[:, 0 : h * F],
                            macc[:, 0 : h * F],
                            macc[:, (cur - h) * F : cur * F],
                        )
                        cur = cur - h
                    nc.scalar.copy(out=hcat[:, 2 * F : 3 * F], in_=macc[:, 0:F])
                    nc.sync.dma_start(hcat[:, 3 * F : 4 * F], ub[n0 : n0 + P, :])

                    h1T = ps.tile([P, P], f32, tag="tps2", space="PSUM")
                    nc.tensor.transpose(
                        out=h1T[:], in_=hcat[:, 0:P], identity=idn[:]
                    )
                    h1s = sb.tile([P, P], f32, tag="h1s")
                    nc.vector.tensor_copy(out=h1s[:], in_=h1T[:])
                    h2T = ps.tile([P, P], f32, tag="tps2", space="PSUM")
                    nc.tensor.transpose(
                        out=h2T[:], in_=hcat[:, P : 2 * P], identity=idn[:]
                    )
                    h2s = sb.tile([P, P], f32, tag="h2s")
                    nc.scalar.copy(out=h2s[:], in_=h2T[:])

                    g2 = ps.tile([HID, P], f32, tag="g2", space="PSUM")
                    nc.tensor.matmul(
                        out=g2[:], lhsT=w3a[:], rhs=h1s[:], start=True, stop=False
                    )
                    nc.tensor.matmul(
                        out=g2[:], lhsT=w3b[:], rhs=h2s[:], start=False, stop=True
                    )
                    g2s = sb.tile([HID, P], f32, tag="g2s")
                    nc.scalar.activation(
                        out=g2s[:], in_=g2[:], func=RELU, bias=b3t[:, :1]
                    )
                    op = ps.tile([F_OUT, P], f32, tag="op", space="PSUM")
                    nc.tensor.matmul(
                        out=op[:], lhsT=w4t[:], rhs=g2s[:], start=True, stop=True
                    )
                    osb = sb.tile([F_OUT, P], f32, tag="osb")
                    nc.scalar.activation(
                        out=osb[:], in_=op[:], func=RELU, bias=b4t[:, :1]
                    )
                    nc.sync.dma_start(outT[:, n0 : n0 + P], osb[:])

    split_multi_waits(nc)
    return nc


def kernel(x, edge_index, edge_attr, u, batch, W1, b1, W2, b2, W3, b3, W4, b4):
    x = np.asarray(x, np.float32)
    edge_index = np.asarray(edge_index)
    edge_attr = np.asarray(edge_attr, np.float32)
    u = np.asarray(u, np.float32)
    batch = np.asarray(batch)
    n_nodes, _ = x.shape
    n_edges = edge_index.shape[1]
    row = np.asarray(edge_index[0], np.int64)
    col = np.asarray(edge_index[1], np.int64)

    # sort edges by destination; shard at node boundaries
    perm = np.argsort(col, kind="stable")
    col_s = col[perm]
    row_s = row[perm]
    ea_s = edge_attr[perm]
    deg = np.bincount(col, minlength=n_nodes).astype(np.int64)
    cum = np.cumsum(deg)  # cum[n] = first edge index of node n+1
    starts = cum - deg

    # node split points: nearest node boundary to c*E/8
    node_splits = [0]
    for c in range(1, N_CORES):
        node_splits.append(int(np.searchsorted(cum, c * n_edges / N_CORES)))
    node_splits.append(n_nodes)
    e_splits = (
        [0]
        + [int(cum[s - 1]) if s > 0 else 0 for s in node_splits[1:-1]]
        + [n_edges]
    )

    e_counts = [e_splits[c + 1] - e_splits[c] for c in range(N_CORES)]
    n_counts = [node_splits[c + 1] - node_splits[c] for c in range(N_CORES)]
    e_pad = ((max(e_counts) + 1 + ET - 1) // ET) * ET
    nodes_pad = ((max(n_counts) + P - 1) // P) * P
    d_max = int(deg.max())
    n_nblocks = nodes_pad // P

    ub_full = u[np.asarray(batch, np.int64)]  # [N, F]

    # per-block regather depth: max over cores of block max degree
    db_list = [1] * n_nblocks
    per_core = []
    for c in range(N_CORES):
        e0, e1 = e_splits[c], e_splits[c + 1]
        nlo, nhi = node_splits[c], node_splits[c + 1]
        ec = e1 - e0
        ncnt = nhi - nlo

        rc = np.zeros(2 * e_pad, np.int32)
        rc[0 : 2 * ec : 2] = row_s[e0:e1]
        rc[1 : 2 * ec + 1 : 2] = col_s[e0:e1]
        msk = np.zeros(e_pad, np.float32)
        msk[:ec] = 1.0
        eac = np.zeros((e_pad, F), np.float32)
        eac[:ec] = ea_s[e0:e1]

        degc = np.zeros(nodes_pad, np.int64)
        degc[:ncnt] = deg[nlo:nhi]
        startc = np.zeros(nodes_pad, np.int64)
        startc[:ncnt] = starts[nlo:nhi] - e0
        dd = np.arange(d_max)[None, :]
        pidx = startc[:, None] + dd
        pidx = np.where(dd < degc[:, None], pidx, e_pad - 1).astype(np.int32)
        cntiv = (1.0 / np.maximum(degc, 1)).astype(np.float32)
        for nb in range(n_nblocks):
            bmax = int(degc[nb * P : (nb + 1) * P].max())
            db_list[nb] = max(db_list[nb], bmax, 1)

        xwc = np.zeros((nodes_pad, F), np.float32)
        xwc[:ncnt] = x[nlo:nhi]
        ubc = np.zeros((nodes_pad, F), np.float32)
        ubc[:ncnt] = ub_full[nlo:nhi]
        per_core.append((rc, msk, eac, pidx, cntiv, xwc, ubc))

    in_maps = []
    for c in range(N_CORES):
        rc, msk, eac, pidx, cntiv, xwc, ubc = per_core[c]
        in_maps.append(
            {
                "x": x,
                "rcT": rc.reshape(-1, P, 2).transpose(1, 0, 2).reshape(P, -1).copy(),
                "maskT": msk.reshape(-1, P).T.copy(),
                "ea": eac,
                "padidx": pidx,
                "cntinvT": cntiv.reshape(n_nblocks, P).T.copy(),
                "xw": xwc,
                "ub": ubc,
                "ident": np.eye(P, dtype=np.float32),
                "w1": np.asarray(W1, np.float32),
                "b1": np.asarray(b1, np.float32).reshape(-1, 1),
                "w2": np.asarray(W2, np.float32),
                "b2": np.asarray(b2, np.float32).reshape(1, -1),
                "w3": np.asarray(W3, np.float32),
                "b3": np.asarray(b3, np.float32).reshape(-1, 1),
                "w4": np.asarray(W4, np.float32),
                "b4": np.asarray(b4, np.float32).reshape(-1, 1),
            }
        )

    b2_nonzero = bool(np.any(np.asarray(b2) != 0))
    nc = build_program(n_nodes, e_pad, nodes_pad, d_max, db_list, b2_nonzero)
    res = run_bass_kernel_spmd(nc, in_maps, list(range(N_CORES)))

    out = np.empty((n_nodes, F_OUT), np.float32)
    for c in range(N_CORES):
        nlo, nhi = node_splits[c], node_splits[c + 1]
        out[nlo:nhi] = res.results[c]["outT"][:, : nhi - nlo].T
    kernel.last = {"nc": nc, "in_maps": in_maps, "node_splits": node_splits}
    return out


# revision 13
# speedup vs baseline: 1.1139x; 1.1139x over previous
"""Trainium2 Bass kernel for the GNN NodeModel problem.

Strategy:
  - Host: sort edges by destination (col), shard contiguously at node
    boundaries across 8 cores (no cross-core combine needed: each core
    owns a disjoint node range and all edges pointing into it).
  - Device phase 1 (edge MLP): one interleaved indirect-DMA gather of
    x[row],x[col] per 128 edges, PE-transpose to feature-major, 2-chunk
    matmul MLP1 (K=192), MLP2, relu+pad-mask, messages to DRAM scratch.
  - Device phase 2 (aggregate + node MLP): per 128-node block, padded
    indirect-DMA regather of messages (per-block max degree), free-dim
    reduce for sum, log-tree max, mean via 1/cnt, concat with x and
    u[batch], PE-transpose, MLP3/MLP4, output feature-major.
  - Three TileContexts (phase1 / phase2a / phase2b) so DMA semaphore
    counters reset — one context overflows the 16-bit wait immediates.
  - Host: concat per-core node ranges, transpose to [N, F_OUT].
"""

import os
import sys

sys.path.insert(0, "/opt/trn_rl_repo")
os.environ.setdefault("NEURON_RT_RESET_CORES", "1")

import numpy as np

import concourse.bass as bass
import concourse.mybir as mybir
import concourse.tile as tile
from concourse.alu_op_type import AluOpType
from concourse.bass import IndirectOffsetOnAxis
from concourse.bass_utils import run_bass_kernel_spmd

N_CORES = 8
F = 64  # node/edge/msg/glob feature dim
HID = 32
F_OUT = 128
ET = 512  # edge tile (moving free dim)
P = 128

f32 = mybir.dt.float32
i32 = mybir.dt.int32
RELU = mybir.ActivationFunctionType.Relu


def split_multi_waits(nc):
    """This walrus build allows max 1 sync-wait per instruction; hoist
    extras onto same-engine NoOps inserted just before."""
    for f in nc.m.functions:
        for bb in f.blocks:
            new_insts = []
            for inst in bb.instructions:
                si = inst.sync_info
                waits = list(si.on_wait) if si and si.on_wait else []
                if len(waits) > 1:
                    for k, w in enumerate(waits[:-1]):
                        nop = mybir.InstNoOp(
                            name=f"{inst.name}-wsplit{k}",
                            engine=inst.engine,
                            ins=[],
                            outs=[],
                        )
                        nop.sync_info = mybir.SyncInfo(on_wait=[w], on_update=[])
                        new_insts.append(nop)
                    inst.sync_info = mybir.SyncInfo(
                        on_wait=[waits[-1]], on_update=list(si.on_update)
                    )
                new_insts.append(inst)
            bb.instructions[:] = new_insts


def build_program(n_nodes_total, e_pad, nodes_pad, d4_max, db_list, b2_nonzero):
    """Build the SPMD Bass program. e_pad: padded edge count per core
    (multiple of ET, with >= 4 trailing pad edges; every node's segment
    is padded to a multiple of 4 so messages are gathered 4-per-index).
    d4_max / db_list: quad-regather depths (global / per node block)."""
    nc = bass.Bass()
    n_etiles = e_pad // ET
    n_nblocks = nodes_pad // P
    t4 = n_etiles * 4  # 128-edge groups
    assert len(db_list) == n_nblocks

    x = nc.dram_tensor("x", [n_nodes_total, F], f32, kind="ExternalInput")
    rcT = nc.dram_tensor("rcT", [P, 2 * t4], i32, kind="ExternalInput")
    maskT = nc.dram_tensor("maskT", [P, t4], f32, kind="ExternalInput")
    ea = nc.dram_tensor("ea", [e_pad, F], f32, kind="ExternalInput")
    padidx = nc.dram_tensor("padidx", [nodes_pad, d4_max], i32, kind="ExternalInput")
    cntinvT = nc.dram_tensor("cntinvT", [P, n_nblocks], f32, kind="ExternalInput")
    xw = nc.dram_tensor("xw", [nodes_pad, F], f32, kind="ExternalInput")
    ub = nc.dram_tensor("ub", [nodes_pad, F], f32, kind="ExternalInput")
    ident = nc.dram_tensor("ident", [P, P], f32, kind="ExternalInput")
    w1 = nc.dram_tensor("w1", [3 * F, HID], f32, kind="ExternalInput")
    b1 = nc.dram_tensor("b1", [HID, 1], f32, kind="ExternalInput")
    w2 = nc.dram_tensor("w2", [HID, F], f32, kind="ExternalInput")
    b2 = nc.dram_tensor("b2", [1, F], f32, kind="ExternalInput")
    w3 = nc.dram_tensor("w3", [4 * F, HID], f32, kind="ExternalInput")
    b3 = nc.dram_tensor("b3", [HID, 1], f32, kind="ExternalInput")
    w4 = nc.dram_tensor("w4", [HID, F_OUT], f32, kind="ExternalInput")
    b4 = nc.dram_tensor("b4", [F_OUT, 1], f32, kind="ExternalInput")

    outT = nc.dram_tensor("outT", [F_OUT, nodes_pad], f32, kind="ExternalOutput")
    m_dram4 = nc.dram_tensor("m_dram4", [e_pad // 4, 4 * F], f32, kind="Internal")

    # ---------------- phase 1: edge MLP ----------------
    with tile.TileContext(nc) as tc:
        with (
            tc.tile_pool(name="c1", bufs=1) as cpool,
            tc.tile_pool(name="sb1", bufs=3) as sb,
            tc.tile_pool(name="ps1", bufs=2, space="PSUM") as ps,
            tc.tile_pool(name="mps", bufs=2, space="PSUM") as mps,
        ):
            idn = cpool.tile([P, P], f32)
            nc.sync.dma_start(idn[:], ident[:])
            w1a = cpool.tile([2 * F, HID], f32)
            nc.sync.dma_start(w1a[:], w1[0 : 2 * F, :])
            w1b = cpool.tile([F, HID], f32)
            nc.sync.dma_start(w1b[:], w1[2 * F : 3 * F, :])
            b1t = cpool.tile([HID, 1], f32)
            nc.sync.dma_start(b1t[:], b1[:])
            w2t = cpool.tile([HID, F], f32)
            nc.sync.dma_start(w2t[:], w2[:])
            b2t = cpool.tile([1, F], f32)
            nc.sync.dma_start(b2t[:], b2[:])
            rcTt = cpool.tile([P, 2 * t4], i32)
            nc.sync.dma_start(rcTt[:], rcT[:])
            maskTt = cpool.tile([P, t4], f32)
            nc.sync.dma_start(maskTt[:], maskT[:])

            for it in range(n_etiles):
                e0 = it * ET
                xallT = sb.tile([P, ET], f32, tag="xallT")
                eaT = sb.tile([F, ET], f32, tag="eaT")
                for j in range(4):
                    gidx = 4 * it + j
                    g = sb.tile([P, 2 * F], f32, tag="g")
                    nc.gpsimd.indirect_dma_start(
                        out=g[:, 0:F],
                        out_offset=None,
                        in_=x[:, :],
                        in_offset=IndirectOffsetOnAxis(
                            ap=rcTt[:, 2 * gidx : 2 * gidx + 1], axis=0
                        ),
                    )
                    nc.gpsimd.indirect_dma_start(
                        out=g[:, F : 2 * F],
                        out_offset=None,
                        in_=x[:, :],
                        in_offset=IndirectOffsetOnAxis(
                            ap=rcTt[:, 2 * gidx + 1 : 2 * gidx + 2], axis=0
                        ),
                    )
                    gT = ps.tile([P, P], f32, tag="tps", space="PSUM")
                    nc.tensor.transpose(out=gT[:], in_=g[:], identity=idn[:])
                    if j % 2 == 0:
                        nc.vector.tensor_copy(
                            out=xallT[:, j * P : (j + 1) * P], in_=gT[:]
                        )
                    else:
                        nc.scalar.copy(out=xallT[:, j * P : (j + 1) * P], in_=gT[:])

                    gea = sb.tile([P, F], f32, tag="gea")
                    nc.sync.dma_start(gea[:], ea[e0 + j * P : e0 + (j + 1) * P, :])
                    geaT = ps.tile([P, P], f32, tag="tps", space="PSUM")
                    nc.tensor.transpose(out=geaT[:F, :], in_=gea[:], identity=idn[:])
                    if j % 2 == 0:
                        nc.scalar.copy(out=eaT[:, j * P : (j + 1) * P], in_=geaT[:F, :])
                    else:
                        nc.vector.tensor_copy(
                            out=eaT[:, j * P : (j + 1) * P], in_=geaT[:F, :]
                        )

                hps = ps.tile([HID, ET], f32, tag="hps", space="PSUM")
                nc.tensor.matmul(
                    out=hps[:], lhsT=w1a[:], rhs=xallT[:], start=True, stop=False
                )
                nc.tensor.matmul(
                    out=hps[:], lhsT=w1b[:], rhs=eaT[:], start=False, stop=True
                )
                hsb = sb.tile([HID, ET], f32, tag="hsb")
                nc.scalar.activation(
                    out=hsb[:], in_=hps[:], func=RELU, bias=b1t[:, :1]
                )
                for j in range(4):
                    mp = mps.tile([P, F], f32, tag="mp", space="PSUM")
                    nc.tensor.matmul(
                        out=mp[:],
                        lhsT=hsb[:, j * P : (j + 1) * P],
                        rhs=w2t[:],
                        start=True,
                        stop=True,
                    )
                    msb = sb.tile([P, F], f32, tag="msb")
                    if b2_nonzero:
                        madd = sb.tile([P, F], f32, tag="madd")
                        nc.vector.tensor_add(
                            madd[:], mp[:], b2t[:].to_broadcast([P, F])
                        )
                        nc.scalar.activation(out=msb[:], in_=madd[:], func=RELU)
                    else:
                        nc.scalar.activation(out=msb[:], in_=mp[:], func=RELU)
                    mz = sb.tile([P, F], f32, tag="mz")
                    nc.vector.tensor_scalar_mul(
                        mz[:], msb[:], maskTt[:, 4 * it + j : 4 * it + j + 1]
                    )
                    q0 = (e0 + j * P) // 4
                    nc.sync.dma_start(
                        m_dram4[q0 : q0 + P // 4, :].rearrange(
                            "q (k f) -> (q k) f", k=4
                        ),
                        mz[:],
                    )

    # ---------------- phase 2: aggregate + node MLP (two contexts) ----
    halves = [(0, n_nblocks // 2), (n_nblocks // 2, n_nblocks)]
    for lo, hi in halves:
        if lo == hi:
            continue
        with tile.TileContext(nc) as tc:
            with (
                tc.tile_pool(name=f"c2_{lo}", bufs=1) as cpool,
                tc.tile_pool(name=f"sb2_{lo}", bufs=3) as sb,
                tc.tile_pool(name=f"ps2_{lo}", bufs=2, space="PSUM") as ps,
            ):
                idn = cpool.tile([P, P], f32)
                nc.sync.dma_start(idn[:], ident[:])
                w3a = cpool.tile([2 * F, HID], f32)
                nc.sync.dma_start(w3a[:], w3[0 : 2 * F, :])
                w3b = cpool.tile([2 * F, HID], f32)
                nc.sync.dma_start(w3b[:], w3[2 * F : 4 * F, :])
                b3t = cpool.tile([HID, 1], f32)
                nc.sync.dma_start(b3t[:], b3[:])
                w4t = cpool.tile([HID, F_OUT], f32)
                nc.sync.dma_start(w4t[:], w4[:])
                b4t = cpool.tile([F_OUT, 1], f32)
                nc.sync.dma_start(b4t[:], b4[:])
                cntTt = cpool.tile([P, n_nblocks], f32)
                nc.sync.dma_start(cntTt[:], cntinvT[:])

                for nb in range(lo, hi):
                    n0 = nb * P
                    db = db_list[nb]
                    pidx = sb.tile([P, d4_max], i32, tag="pidx")
                    nc.sync.dma_start(pidx[:, :db], padidx[n0 : n0 + P, 0:db])

                    macc = sb.tile([P, d4_max * 4 * F], f32, tag="macc")
                    for d in range(db):
                        nc.gpsimd.indirect_dma_start(
                            out=macc[:, d * 4 * F : (d + 1) * 4 * F],
                            out_offset=None,
                            in_=m_dram4[:, :],
                            in_offset=IndirectOffsetOnAxis(
                                ap=pidx[:, d : d + 1], axis=0
                            ),
                        )
                    mview = macc[:, 0 : db * 4 * F].rearrange(
                        "p (d f) -> p f d", f=F
                    )
                    hcat = sb.tile([P, 4 * F], f32, tag="hcat")
                    nc.sync.dma_start(hcat[:, 0:F], xw[n0 : n0 + P, :])
                    asum = sb.tile([P, F], f32, tag="asum")
                    nc.vector.tensor_reduce(
                        out=asum[:], in_=mview, axis=mybir.AxisListType.X,
                        op=AluOpType.add,
                    )
                    nc.vector.tensor_scalar_mul(
                        hcat[:, F : 2 * F], asum[:], cntTt[:, nb : nb + 1]
                    )
                    # segment max: log-tree of elementwise max (DVE),
                    # runs after the sum reduce via WAR dependency
                    cur = db * 4
                    while cur > 1:
                        h = cur // 2
                        nc.vector.tensor_max(
                            macc[:, 0 : h * F],
                            macc[:, 0 : h * F],
                            macc[:, (cur - h) * F : cur * F],
                        )
                        cur = cur - h
                    nc.scalar.copy(out=hcat[:, 2 * F : 3 * F], in_=macc[:, 0:F])
                    nc.sync.dma_start(hcat[:, 3 * F : 4 * F], ub[n0 : n0 + P, :])

                    h1T = ps.tile([P, P], f32, tag="tps2", space="PSUM")
                    nc.tensor.transpose(
                        out=h1T[:], in_=hcat[:, 0:P], identity=idn[:]
                    )
                    h1s = sb.tile([P, P], f32, tag="h1s")
                    nc.vector.tensor_copy(out=h1s[:], in_=h1T[:])
                    h2T = ps.tile([P, P], f32, tag="tps2", space="PSUM")
                    nc.tensor.transpose(
                        out=h2T[:], in_=hcat[:, P : 2 * P], identity=idn[:]
                    )
                    h2s = sb.tile([P, P], f32, tag="h2s")
                    nc.scalar.copy(out=h2s[:], in_=h2T[:])

                    g2 = ps.tile([HID, P], f32, tag="g2", space="PSUM")
                    nc.tensor.matmul(
                        out=g2[:], lhsT=w3a[:], rhs=h1s[:], start=True, stop=False
                    )
                    nc.tensor.matmul(
                        out=g2[:], lhsT=w3b[:], rhs=h2s[:], start=False, stop=True
                    )
                    g2s = sb.tile([HID, P], f32, tag="g2s")
                    nc.scalar.activation(
                        out=g2s[:], in_=g2[:], func=RELU, bias=b3t[:, :1]
                    )
                    op = ps.tile([F_OUT, P], f32, tag="op", space="PSUM")
                    nc.tensor.matmul(
                        out=op[:], lhsT=w4t[:], rhs=g2s[:], start=True, stop=True
                    )
                    osb = sb.tile([F_OUT, P], f32, tag="osb")
                    nc.scalar.activation(
                        out=osb[:], in_=op[:], func=RELU, bias=b4t[:, :1]
                    )
                    nc.sync.dma_start(outT[:, n0 : n0 + P], osb[:])

    split_multi_waits(nc)
    return nc


def kernel(x, edge_index, edge_attr, u, batch, W1, b1, W2, b2, W3, b3, W4, b4):
    x = np.asarray(x, np.float32)
    edge_index = np.asarray(edge_index)
    edge_attr = np.asarray(edge_attr, np.float32)
    u = np.asarray(u, np.float32)
    batch = np.asarray(batch)
    n_nodes, _ = x.shape
    n_edges = edge_index.shape[1]
    row = np.asarray(edge_index[0], np.int64)
    col = np.asarray(edge_index[1], np.int64)

    # sort edges by destination; shard at node boundaries
    perm = np.argsort(col, kind="stable")
    col_s = col[perm]
    row_s = row[perm]
    ea_s = edge_attr[perm]
    deg = np.bincount(col, minlength=n_nodes).astype(np.int64)
    cum = np.cumsum(deg)  # cum[n] = first edge index of node n+1
    starts = cum - deg

    # node split points: nearest node boundary to c*E/8
    node_splits = [0]
    for c in range(1, N_CORES):
        node_splits.append(int(np.searchsorted(cum, c * n_edges / N_CORES)))
    node_splits.append(n_nodes)
    e_splits = (
        [0]
        + [int(cum[s - 1]) if s > 0 else 0 for s in node_splits[1:-1]]
        + [n_edges]
    )

    n_counts = [node_splits[c + 1] - node_splits[c] for c in range(N_CORES)]
    nodes_pad = ((max(n_counts) + P - 1) // P) * P
    n_nblocks = nodes_pad // P

    # pad every node's segment to a multiple of 4 so phase 2 can gather
    # 4 messages (256 contiguous floats) per index
    deg4_all = np.where(deg > 0, ((deg + 3) // 4) * 4, 0)
    e4_counts = [
        int(deg4_all[node_splits[c] : node_splits[c + 1]].sum())
        for c in range(N_CORES)
    ]
    e_pad = ((max(e4_counts) + 4 + ET - 1) // ET) * ET
    d4_max = int(((deg.max() + 3) // 4))
    zero_quad = e_pad // 4 - 1

    ub_full = u[np.asarray(batch, np.int64)]  # [N, F]

    # per-block regather depth (quads): max over cores of block max
    db_list = [1] * n_nblocks
    per_core = []
    for c in range(N_CORES):
        e0, e1 = e_splits[c], e_splits[c + 1]
        nlo, nhi = node_splits[c], node_splits[c + 1]
        ec = e1 - e0
        ncnt = nhi - nlo

        degc = np.zeros(nodes_pad, np.int64)
        degc[:ncnt] = deg[nlo:nhi]
        deg4c = np.zeros(nodes_pad, np.int64)
        deg4c[:ncnt] = deg4_all[nlo:nhi]
        start4 = np.cumsum(deg4c) - deg4c  # padded-layout segment starts

        # positions of real edges in the padded layout
        node_loc = (col_s[e0:e1] - nlo).astype(np.int64)
        startc_real = starts[nlo:nhi] - e0
        rank = np.arange(ec, dtype=np.int64) - startc_real[node_loc]
        pos = (start4[node_loc] + rank).astype(np.int64)

        rc = np.zeros(2 * e_pad, np.int32)
        rc[2 * pos] = row_s[e0:e1]
        rc[2 * pos + 1] = col_s[e0:e1]
        msk = np.zeros(e_pad, np.float32)
        msk[pos] = 1.0
        eac = np.zeros((e_pad, F), np.float32)
        eac[pos] = ea_s[e0:e1]

        dd = np.arange(d4_max)[None, :]
        pidx = start4[:, None] // 4 + dd
        pidx = np.where(dd < deg4c[:, None] // 4, pidx, zero_quad).astype(np.int32)
        cntiv = (1.0 / np.maximum(degc, 1)).astype(np.float32)
        for nb in range(n_nblocks):
            bmax = int(deg4c[nb * P : (nb + 1) * P].max() // 4)
            db_list[nb] = max(db_list[nb], bmax, 1)

        xwc = np.zeros((nodes_pad, F), np.float32)
        xwc[:ncnt] = x[nlo:nhi]
        ubc = np.zeros((nodes_pad, F), np.float32)
        ubc[:ncnt] = ub_full[nlo:nhi]
        per_core.append((rc, msk, eac, pidx, cntiv, xwc, ubc))

    in_maps = []
    for c in range(N_CORES):
        rc, msk, eac, pidx, cntiv, xwc, ubc = per_core[c]
        in_maps.append(
            {
                "x": x,
                "rcT": rc.reshape(-1, P, 2).transpose(1, 0, 2).reshape(P, -1).copy(),
                "maskT": msk.reshape(-1, P).T.copy(),
                "ea": eac,
                "padidx": pidx,
                "cntinvT": cntiv.reshape(n_nblocks, P).T.copy(),
                "xw": xwc,
                "ub": ubc,
                "ident": np.eye(P, dtype=np.float32),
                "w1": np.asarray(W1, np.float32),
                "b1": np.asarray(b1, np.float32).reshape(-1, 1),
                "w2": np.asarray(W2, np.float32),
                "b2": np.asarray(b2, np.float32).reshape(1, -1),
                "w3": np.asarray(W3, np.float32),
                "b3": np.asarray(b3, np.float32).reshape(-1, 1),
                "w4": np.asarray(W4, np.float32),
                "b4": np.asarray(b4, np.float32).reshape(-1, 1),
            }
        )

    b2_nonzero = bool(np.any(np.asarray(b2) != 0))
    nc = build_program(n_nodes, e_pad, nodes_pad, d4_max, db_list, b2_nonzero)
    res = run_bass_kernel_spmd(nc, in_maps, list(range(N_CORES)))

    out = np.empty((n_nodes, F_OUT), np.float32)
    for c in range(N_CORES):
        nlo, nhi = node_splits[c], node_splits[c + 1]
        out[nlo:nhi] = res.results[c]["outT"][:, : nhi - nlo].T
    kernel.last = {"nc": nc, "in_maps": in_maps, "node_splits": node_splits}
    return out
